# revision 4
# baseline (speedup 1.0000x reference)
"""Trainium2 Bass kernel for nn_Embedding2Score — row-parallel variant.

Sharding (8 NeuronCores), NO collectives:
  - core k owns graphs [k*128, (k+1)*128) == nodes [k*6400, (k+1)*6400).
  - phase 1 (attention + segment pooling) is data-parallel: each core
    computes s_h for its own 128 graphs from its own nodes.
  - z phase is row-parallel: core k computes z[own 128 rows, ALL V
    columns], reading the FULL item table. The item table is stored in
    fp8 e3m4 (4 mantissa bits, range +-15.5 covers N(0,1) items), which
    halves the dominant read to 12.8MB/core and keeps the added
    quantization error ~1.4e-2 rel (budget 2e-2; s_h stays bf16 in the
    mixed-dtype matmul).
  This removes the CC-firmware AllGather whose fixed entry barrier
  (~21->68us) + op latency (~15-30us) + run-to-run variance gated the
  tail of the vocab-parallel variant.

Layout facts (same as the vocab-parallel kernel):
  - All matmuls keep features on partitions so the natural [in,out]
    weight storage is lhsT with zero on-device transposes.
  - alpha is computed as a ROW ([1, N]) via q^T @ S matmuls, broadcast
    to all partitions with gpsimd.partition_broadcast; the ragged
    segment-sum is a DVE windowed reduce (axis X over [H, Bs, L]).
  - z psum drains through double-width (2-bank) psum tiles: one
    1024-col copy per two matmuls (copies are 25-30% cheaper per
    element at 1024 cols, and vec/scalar copies are the 2nd-tightest
    resource after the tensor queue).
  - z is written bf16 (host upcasts); halves the dominant HBM write.
"""

from contextlib import ExitStack

import numpy as np

H = 128
B = 1024
L = 50
N = B * L
V = 100000
M = 8            # cores
Bs = B // M      # 128 graphs / core
Ns = N // M      # 6400 nodes / core
CH = 512         # matmul chunk width
ZG = 6250        # z output group width (vocab cols); 2 groups per shard
NT = 8           # z tiles per core (each Vs = 12500 cols)
Vs = V // NT


def _sigmoid(x):
    out = np.empty_like(x)
    np.negative(x, out=out)
    np.exp(out, out=out)
    out += 1.0
    np.reciprocal(out, out=out)
    return out


def _kernel_numpy(session, item, batch, W1, b1, W2, b2, q, bq, W3, b3):
    """General-batch fallback (host only). Handles any sorted batch."""
    nb = int(batch.max()) + 1
    last_idx = np.searchsorted(batch, np.arange(nb), side="right") - 1
    v_n = session[last_idx]
    pre = _sigmoid(v_n[batch] @ W1 + b1 + session @ W2 + b2)
    alpha = pre @ q + bq
    w = alpha * session
    s_g = np.zeros((nb, session.shape[1]), np.float32)
    np.add.at(s_g, batch, w)
    s_h = np.concatenate([v_n, s_g], axis=1) @ W3 + b3
    return (s_h @ item.T).astype(np.float32)


def _build_program(bq_val):
    import concourse.bass as bass
    import concourse.bacc as bacc
    import concourse.tile as tile
    from concourse import mybir

    F32 = mybir.dt.float32
    BF16 = mybir.dt.bfloat16
    F8 = mybir.dt.float8e3
    SIG = mybir.ActivationFunctionType.Sigmoid
    IDN = mybir.ActivationFunctionType.Identity

    nc = bacc.Bacc("TRN2", target_bir_lowering=False, debug=False,
                   num_devices=M)

    # ---- DRAM I/O (per-core data; identical program on all cores) ----
    d_xT = nc.dram_tensor("xT", [H, Ns], BF16, kind="ExternalInput").ap()
    # v_n^T for OWN graphs only
    d_vn = nc.dram_tensor("vn", [H, Bs], BF16, kind="ExternalInput").ap()
    # bf16 weights packed: [W1 | W2 | W3a | W3b | q] along the free dim
    d_wp = nc.dram_tensor("wp", [H, 4 * H + 1], BF16,
                          kind="ExternalInput").ap()
    # f32 biases packed: [b1+b2 | b3]
    d_bp = nc.dram_tensor("bp", [H, 2], F32, kind="ExternalInput").ap()
    # FULL item table, fp8 e3m4
    d_item = nc.dram_tensor("itemT", [H, V], F8, kind="ExternalInput").ap()
    # z rows for OWN graphs x full vocab
    d_z = nc.dram_tensor("z", [Bs, V], BF16, kind="ExternalOutput").ap()

    with tile.TileContext(nc) as tc, ExitStack() as ctx:
        nc_ = tc.nc

        consts = ctx.enter_context(tc.tile_pool(name="consts", bufs=1))
        small = ctx.enter_context(tc.tile_pool(name="small", bufs=1))
        item_pool = ctx.enter_context(tc.tile_pool(name="itemp", bufs=1))
        work = ctx.enter_context(tc.tile_pool(name="work", bufs=3))
        big1 = ctx.enter_context(tc.tile_pool(name="big1", bufs=1))
        zout = ctx.enter_context(tc.tile_pool(name="zout", bufs=4))
        psum_a = ctx.enter_context(
            tc.tile_pool(name="psum_a", bufs=2, space="PSUM"))
        # psum_q lives only through phase 1a (scoped below); its 2 banks
        # are then reused by a third double-width psum_z buffer.

        # ---- input loads ----
        wp_sb = consts.tile([H, 4 * H + 1], BF16)
        bp_sb = consts.tile([H, 2], F32)
        vn_sb = consts.tile([H, Bs], BF16)
        xT_sb = big1.tile([H, Ns], BF16)
        itemT_sb = item_pool.tile([H, V], F8)

        # Urgent phase-1 inputs lead; xT rides the scalar queue (4
        # descriptors), item slices split sync/scalar (4+4 of 1.6MB,
        # scalar stays at 8 outstanding < the ~9-sem wrap limit), rest
        # of item on sync.
        nc_.sync.dma_start(out=wp_sb[:], in_=d_wp[:])
        nc_.sync.dma_start(out=bp_sb[:], in_=d_bp[:])
        nc_.sync.dma_start(out=vn_sb[:], in_=d_vn[:])
        qx = Ns // 4
        for x0 in range(0, Ns, qx):
            nc_.scalar.dma_start(out=xT_sb[:, x0:x0 + qx],
                                 in_=d_xT[:, x0:x0 + qx])
        ISL = 12500         # item descriptor slice (1.6MB fp8)
        # even slices load on sync now; odd slices are issued from the
        # scalar queue only AFTER phase 1a is emitted — a descriptor
        # issue ahead of the sigmoids blocks them (and through MMq, the
        # tensor queue) for ~2.7us, and those slices aren't needed until
        # their z tile (~60us+) anyway.
        deferred_item = []
        for si, c0 in enumerate(range(0, V, ISL)):
            c1 = min(c0 + ISL, V)
            if si % 2 == 1:
                deferred_item.append((c0, c1))
            else:
                nc_.sync.dma_start(out=itemT_sb[:, c0:c1],
                                   in_=d_item[:, c0:c1])

        w1s = wp_sb[:, 0 * H:1 * H]
        w2s = wp_sb[:, 1 * H:2 * H]
        w3as = wp_sb[:, 2 * H:3 * H]
        w3bs = wp_sb[:, 3 * H:4 * H]
        qs = wp_sb[:, 4 * H:4 * H + 1]
        bcs = bp_sb[:, 0:1]
        b3s = bp_sb[:, 1:2]

        # ---- phase 1 prologue: Av^T + bc, broadcast to nodes ----
        av_sb = small.tile([H, Bs], BF16)       # (v_n W1 + bc)^T, own graphs
        avrep = big1.tile([H, Ns], BF16)        # column g repeated L times
        p_av = psum_a.tile([H, CH], F32, tag="pp", name="p_av")
        nc_.tensor.matmul(p_av[:, :Bs], lhsT=w1s, rhs=vn_sb[:],
                          start=True, stop=True)
        nc_.scalar.activation(av_sb[:], p_av[:, :Bs], IDN, bias=bcs)

        AVQ = Ns // 4           # avrep broadcast quarter (1600 cols)

        def avrep_quarter(qi):
            # emitted interleaved with the chunk loop so the first adds
            # don't sit behind one 3.5us full-width broadcast at the
            # head of the vector queue.
            g0, g1 = qi * (Bs // 4), (qi + 1) * (Bs // 4)
            nc_.vector.tensor_copy(
                out=avrep[:, qi * AVQ:(qi + 1) * AVQ].rearrange(
                    "h (g l) -> h g l", g=Bs // 4, l=L),
                in_=av_sb[:, g0:g1].unsqueeze(2).broadcast_to(
                    [H, Bs // 4, L]))

        avrep_quarter(0)

        # ---- phase 1a: S = sigmoid(W2^T X^T + avrep); alpha row = q^T S
        # Chunks are emitted in PAIRS so the tensor queue runs
        # MM1,MM1,MMq,MMq — a lone MMq between MM1s would serialize the
        # whole add->sigmoid->q chain into the matmul cadence.
        alpharow = small.tile([1, Ns], BF16)
        n_chunks = (Ns + CH - 1) // CH
        with tc.tile_pool(name="psum_q", bufs=2, space="PSUM") as psum_q:
            c = 0
            while c < n_chunks:
                # quarter q of avrep must be in place before the first
                # chunk whose span crosses q*1600 cols
                if c == 2:
                    avrep_quarter(1)
                elif c == 6:
                    avrep_quarter(2)
                elif c == 8:
                    avrep_quarter(3)
                pair = []
                for cc2 in (c, c + 1):
                    if cc2 >= n_chunks:
                        continue
                    c0 = cc2 * CH
                    cw = min(CH, Ns - c0)
                    pp = psum_a.tile([H, CH], F32, tag="pp")
                    nc_.tensor.matmul(pp[:, :cw], lhsT=w2s,
                                      rhs=xT_sb[:, c0:c0 + cw],
                                      start=True, stop=True)
                    s_sb = work.tile([H, CH], BF16, tag="schunk")
                    nc_.vector.tensor_add(s_sb[:, :cw], pp[:, :cw],
                                          avrep[:, c0:c0 + cw])
                    nc_.scalar.activation(s_sb[:, :cw], s_sb[:, :cw], SIG)
                    pair.append((c0, cw, s_sb))
                for c0, cw, s_sb in pair:
                    pq = psum_q.tile([1, CH], F32, tag="pq")
                    nc_.tensor.matmul(pq[:1, :cw], lhsT=qs, rhs=s_sb[:, :cw],
                                      start=True, stop=True)
                    nc_.vector.tensor_scalar_add(alpharow[:, c0:c0 + cw],
                                                 pq[:1, :cw], float(bq_val))
                c += 2
        for c0, c1 in deferred_item:
            nc_.scalar.dma_start(out=itemT_sb[:, c0:c1], in_=d_item[:, c0:c1])
        # 3 double-width (2-bank) psum_z buffers: enough pipeline depth
        # that the tensor queue never waits on a draining copy, while
        # 1024-wide copies run ~25-30% cheaper per element on the
        # copy-bound vector/scalar engines.
        psum_z = ctx.enter_context(
            tc.tile_pool(name="psum_z", bufs=3, space="PSUM"))

        # ---- phase 1c: Xw = X^T * alpha; s_g^T = windowed sum over L ----
        alpharep = avrep        # broadcast in place; avrep region q is
        xw_sb = xT_sb           # dead once quarter q's adds ran, and xT
                                # is dead after the in-place multiply
        sgf = small.tile([H, Bs], F32)
        sg_sb = small.tile([H, Bs], BF16)
        qq = Ns // 8
        for qi in range(8):
            s0, s1 = qi * qq, (qi + 1) * qq
            nc_.gpsimd.partition_broadcast(alpharep[:, s0:s1],
                                           alpharow[:, s0:s1])
            nc_.vector.tensor_mul(xw_sb[:, s0:s1], xT_sb[:, s0:s1],
                                  alpharep[:, s0:s1])
            nc_.vector.tensor_reduce(
                out=sgf[:, s0 // L:s1 // L],
                in_=xw_sb[:, s0:s1].rearrange("h (g l) -> h g l", l=L),
                axis=mybir.AxisListType.X, op=mybir.AluOpType.add)
        nc_.vector.tensor_copy(out=sg_sb[:], in_=sgf[:])

        # ---- s_h for own graphs ----
        shT_sb = small.tile([H, Bs], BF16)
        p_sh = psum_a.tile([H, CH], F32, tag="pp", name="p_sh")
        nc_.tensor.matmul(p_sh[:, :Bs], lhsT=w3as, rhs=vn_sb[:],
                          start=True, stop=False)
        nc_.tensor.matmul(p_sh[:, :Bs], lhsT=w3bs, rhs=sg_sb[:],
                          start=False, stop=True)
        nc_.scalar.activation(shT_sb[:], p_sh[:, :Bs], IDN, bias=b3s)

        eng_i = 0

        def z_tile(isrc, dst, ring_sel, fine=False):
            """[128 own rows] x 2 groups of ZG cols from item cols isrc.
            fine=True drains the last group per-copy (~1024 cols) so the
            kernel's final store is never an exposed ~0.8MB."""
            nonlocal eng_i
            FS = 3072  # copy-aligned split point within a group
            for g in (0, 1):
                g0 = g * ZG
                zt = zout.tile([H, ZG], BF16, tag="zt")
                drained = 0
                for u in range(0, ZG, 2 * CH):
                    uw = min(2 * CH, ZG - u)
                    zp = psum_z.tile([H, 2 * CH], F32, tag="zp")
                    for v in range(0, uw, CH):
                        vw = min(CH, uw - v)
                        nc_.tensor.matmul(
                            zp[:, v:v + vw], lhsT=shT_sb[:],
                            rhs=itemT_sb[:, isrc + g0 + u + v:
                                         isrc + g0 + u + v + vw],
                            start=True, stop=True)
                    if eng_i % 2 == 0:
                        nc_.vector.tensor_copy(out=zt[:, u:u + uw],
                                               in_=zp[:, :uw])
                    else:
                        nc_.scalar.copy(out=zt[:, u:u + uw], in_=zp[:, :uw])
                    eng_i += 1
                    if fine and g == 1:
                        ring = nc_.gpsimd if ring_sel % 2 == 0 else nc_.sync
                        ring.dma_start(out=dst[:, g0 + u:g0 + u + uw],
                                       in_=zt[:, u:u + uw])
                        ring_sel += 1
                        drained = u + uw
                    elif u + uw == FS:
                        ring = nc_.gpsimd if ring_sel % 2 == 0 else nc_.sync
                        ring.dma_start(out=dst[:, g0:g0 + FS],
                                       in_=zt[:, :FS])
                        ring_sel += 1
                        drained = FS
                if drained < ZG:
                    ring = nc_.gpsimd if ring_sel % 2 == 0 else nc_.sync
                    ring.dma_start(out=dst[:, g0 + drained:g0 + ZG],
                                   in_=zt[:, drained:])
                    ring_sel += 1

        # consume sync-ring item slices (loaded by ~40us) before the
        # deferred scalar-ring ones (issued only after phase 1a, landing
        # ~45-62us) so no early z tile races its slice's arrival.
        for i, t in enumerate((0, 2, 4, 6, 1, 3, 5, 7)):
            z_tile(t * Vs, d_z[:, t * Vs:(t + 1) * Vs], i,
                   fine=(i == NT - 1))

    nc.compile()
    return nc


_CACHE = {}


def _get_program(bq_val):
    key = round(float(bq_val), 10)
    if key not in _CACHE:
        _CACHE[key] = _build_program(bq_val)
    return _CACHE[key]


def kernel(session_embedding, item_emb, batch, num_graphs,
           W1, b1, W2, b2, q, bq, W3, b3):
    import ml_dtypes
    BF = ml_dtypes.bfloat16
    F8NP = ml_dtypes.float8_e3m4

    session = np.ascontiguousarray(np.asarray(session_embedding, np.float32))
    item = np.ascontiguousarray(np.asarray(item_emb, np.float32))
    batch = np.asarray(batch)
    W1 = np.asarray(W1, np.float32)
    b1 = np.asarray(b1, np.float32)
    W2 = np.asarray(W2, np.float32)
    b2 = np.asarray(b2, np.float32)
    q = np.asarray(q, np.float32)
    bq = np.asarray(bq, np.float32)
    W3 = np.asarray(W3, np.float32)
    b3 = np.asarray(b3, np.float32)

    uniform = (session.shape == (N, H) and item.shape == (V, H)
               and batch.shape == (N,)
               and int(num_graphs) == B
               and np.array_equal(batch, np.repeat(np.arange(B), L)))
    if not uniform:
        return _kernel_numpy(session, item, batch, W1, b1, W2, b2,
                             q, bq, W3, b3)

    from concourse.bass_utils import run_bass_kernel_spmd

    nc = _get_program(bq[0])

    # ---- host-side shard prep (index bookkeeping + casts) ----
    last_idx = np.arange(B) * L + (L - 1)
    v_n = session[last_idx]                          # [B, H]
    vnT = np.ascontiguousarray(v_n.T.astype(BF))     # [H, B]

    itemT = np.ascontiguousarray(item.T.astype(F8NP))  # [H, V] fp8 e3m4
    sessT = session.T.astype(BF)                       # [H, N]

    wp = np.concatenate(
        [W1, W2, W3[:H], W3[H:], q.reshape(H, 1)], axis=1).astype(BF)
    wp = np.ascontiguousarray(wp)
    bp = np.ascontiguousarray(
        np.stack([b1 + b2, b3], axis=1).astype(np.float32))

    in_maps = []
    for k in range(M):
        nsl = slice(k * Ns, (k + 1) * Ns)
        in_maps.append({
            "xT": np.ascontiguousarray(sessT[:, nsl]),
            "vn": np.ascontiguousarray(vnT[:, k * Bs:(k + 1) * Bs]),
            "itemT": itemT,
            "wp": wp, "bp": bp,
        })

    res = run_bass_kernel_spmd(nc, in_maps, list(range(M)))

    z = np.empty((B, V), np.float32)
    for k in range(M):
        z[k * Bs:(k + 1) * Bs, :] = res.results[k]["z"].astype(np.float32)
    return z
